# revision 26
# baseline (speedup 1.0000x reference)
"""DIN activation unit kernel for 8x TRN2 NeuronCores (V2.2).

Math (per batch row b, per key position t):
  h[t]  = W_b @ k[t] + c_b,  W_b = (Wk-Wc) + Wd*diag(q_b)  (host-folded)
  s[t]  = w2 . PReLU(h[t], 0.25)
  p     = softmax over masked t;  w = p*mask / max(sum, 1e-6)
  out   = sum_t w[t] * k[t]

Pipeline: 4 half-blocks of 64 rows; each half's scores accumulate in its
own 64-partition PSUM region (own stop flag) so softmax + weighted
reduction of half X overlap the MLP of half X+1.

Per row pair the PSUM tile is seeded with the bias columns by one
selector matmul (stationary = natural-layout bias block, moving = two
eye columns stride-0-replicated across t), the two h matmuls accumulate
on top, and ONE PReLU activation (no bias) covers both rows -- halving
the scalar-engine instruction count, the old wall.

The weighted reduction runs as one bf16 2x-mode tensor_tensor
(kt * broadcast weights) + one tensor_reduce per 16-row chunk. Softmax
weights reach all 128 partitions via DRAM bounce (write on the scalar
HWDGE ring, stride-0 partition-broadcast read on the sync ring).
Softmax skips max-subtraction (scores are small; exp stays in fp32
range). kt/W_b stream on the sync ring in 16-row chunks.
"""

import numpy as np
import ml_dtypes

B, T, D = 2048, 200, 128
NCORES = 8
BC = B // NCORES          # 256 batch rows per core
NHALF = 4                 # 64-row half-blocks per core
NCH = 16                  # 16-row chunks per core (4 per half)
BF16 = ml_dtypes.bfloat16
BIG = 1024.0              # mask shift; exp(-~1024) == 0 in fp32

# processing order within a 64-row half: pairs (k, k+32) so one
# double-diag scores matmul serves both rows
LBH = [32 * (i % 2) + i // 2 for i in range(64)]

_CACHE = {}


def _build_module():
    from contextlib import ExitStack

    import concourse.bacc as bacc
    import concourse.mybir as mybir
    from concourse import tile
    from concourse.bass import AP

    fp32 = mybir.dt.float32
    bf16 = mybir.dt.bfloat16
    Alu = mybir.AluOpType
    AF = mybir.ActivationFunctionType

    nc = bacc.Bacc(
        "TRN2", target_bir_lowering=False, debug=False, num_devices=NCORES
    )

    # transposed keys, [d, t] per row, rows processing-ordered, 16-row chunks
    kt_d = nc.dram_tensor("ktg", [NCH, D, 16, T], bf16, kind="ExternalInput")
    # host-folded W_b per row, [d_in, d_out], same order/chunking
    wb_d = nc.dram_tensor("wbg", [NCH, D, 16, D], bf16, kind="ExternalInput")
    mf_d = nc.dram_tensor("mf", [BC, T], bf16, kind="ExternalInput")
    # bias rows in processing order, natural [row, d] layout
    bt_d = nc.dram_tensor("btn", [2, 128, D], bf16, kind="ExternalInput")
    # precomputed scores selector: wq[d, c, c] = wq[d, c, 32+c] = w2[d]
    wq_d = nc.dram_tensor("wq", [D, 32, 64], bf16, kind="ExternalInput")
    ey_d = nc.dram_tensor("ey", [D, 128], bf16, kind="ExternalInput")
    # processing-order output [half, d, pos]; host undoes the permutation
    out_d = nc.dram_tensor("out", [NHALF, D, 64], fp32, kind="ExternalOutput")

    kt = kt_d.ap()
    wb = wb_d.ap()
    mf = mf_d.ap()
    bt = bt_d.ap()
    out = out_d.ap()

    with ExitStack() as ctx:
        tc = ctx.enter_context(tile.TileContext(nc))
        const = ctx.enter_context(tc.tile_pool(name="const", bufs=1))
        ktp = ctx.enter_context(tc.tile_pool(name="ktp", bufs=12))
        wbp = ctx.enter_context(tc.tile_pool(name="wbp", bufs=5))
        hap = ctx.enter_context(tc.tile_pool(name="hap", bufs=6))
        blkp = ctx.enter_context(tc.tile_pool(name="blkp", bufs=2))
        halfp = ctx.enter_context(tc.tile_pool(name="halfp", bufs=2))
        smallp = ctx.enter_context(tc.tile_pool(name="smallp", bufs=4))
        pbp = ctx.enter_context(tc.tile_pool(name="pbp", bufs=2))
        jkp = ctx.enter_context(tc.tile_pool(name="jkp", bufs=2))
        dramp = ctx.enter_context(tc.tile_pool(name="dramp", bufs=2,
                                               space="DRAM"))
        hpp = ctx.enter_context(tc.tile_pool(name="hpp", bufs=3, space="PSUM"))
        spp = ctx.enter_context(tc.tile_pool(name="spp", bufs=2, space="PSUM"))

        kts = [None] * NCH
        wbs = [None] * NCH

        def load_chunk(g):
            ktt = ktp.tile([D, 16, T], bf16, name="ktt", tag="kt")
            nc.sync.dma_start(ktt[:], kt[g])
            kts[g] = ktt
            wbt = wbp.tile([D, 16, D], bf16, name="wbt", tag="wb")
            nc.sync.dma_start(wbt[:], wb[g])
            wbs[g] = wbt

        load_chunk(0)
        load_chunk(1)

        bt_s, mf_s = [], []
        wq_t = const.tile([D, 32, 64], bf16, name="wq_t")
        nc.sync.dma_start(wq_t[:], wq_d.ap()[:])
        ey_t = const.tile([D, 128], bf16, name="ey_t")
        nc.sync.dma_start(ey_t[:], ey_d.ap()[:])
        for blk in range(2):
            bs = blkp.tile([128, D], bf16, name="bt_s", tag="bt_s")
            nc.sync.dma_start(bs[:], bt[blk])
            bt_s.append(bs)
            ms = blkp.tile([128, T], bf16, name="mf_s", tag="mf_s")
            nc.sync.dma_start(ms[:], mf[blk * 128 : (blk + 1) * 128, :])
            mf_s.append(ms)

        # zero tiles for the s_ps has_written init matmuls
        zw_t = const.tile([D, 64], bf16, name="zw_t")
        nc.gpsimd.memset(zw_t[:], 0.0)
        nbig_t = const.tile([128, 1], fp32, name="nbig_t")
        nc.gpsimd.memset(nbig_t[:], -BIG)
        zr_t = const.tile([D, 2 * T], bf16, name="zr_t")
        nc.gpsimd.memset(zr_t[:], 0.0)
        s_ps = []
        for blk in range(2):
            sp = spp.tile([128, 512], fp32, name="s_ps", tag="s_ps")
            for H in range(2):
                nc.tensor.matmul(sp[64 * H : 64 * H + 64, 0 : 2 * T],
                                 zw_t[:], zr_t[:],
                                 tile_position=(0, 64 * H),
                                 start=True, stop=False, skip_group_check=True)
            s_ps.append(sp)

        # per-half output accumulators [d, pos] (processing order)
        vts = [const.tile([D, 64], fp32, name=f"vt_{h}") for h in range(NHALF)]

        ey_ap = ey_t[:]
        ey_pstep = ey_ap.ap[0][0]

        # scores matmuls deferred by one quad so the PE never sits behind
        # the activation of the quad it just produced
        pending_sc = []

        def flush_sc():
            while pending_sc:
                pending_sc.pop(0)()

        def mlp_chunk(hidx, c_h, extra=None):
            blk, H = hidx // 2, hidx % 2
            g = 4 * hidx + c_h
            if g + 2 < NCH and kts[g + 2] is None:
                load_chunk(g + 2)
            if extra is not None:
                extra()
            ktt, wbt = kts[g], wbs[g]
            for i4 in range(4):                    # quad of 4 rows
                pos0 = 16 * c_h + 4 * i4           # within-half quad base
                posb = 64 * H + pos0               # block-wise row
                # two-bank PSUM tile; row j lives at columns
                # {0:200, 200:400, 512:712, 712:912} so each matmul's
                # output stays inside one bank
                hp4 = hpp.tile([128, 1024], fp32, name="hp4", tag="hp")
                # seed PSUM with bias columns: stationary is the natural
                # bias block, moving = eye cols stride-0-replicated T times
                for q in range(2):
                    esel = AP(ey_ap.tensor, ey_ap.offset + posb + 2 * q,
                              [(ey_pstep, 128), (1, 2), (0, T)])
                    nc.tensor.matmul(hp4[:, 512 * q : 512 * q + 2 * T],
                                     bt_s[blk][:], esel,
                                     start=True, stop=False,
                                     skip_group_check=True)
                for j in range(4):
                    i = 4 * i4 + j
                    lo = 512 * (j // 2) + T * (j % 2)
                    nc.tensor.matmul(hp4[:, lo : lo + T], wbt[:, i, :],
                                     ktt[:, i, :],
                                     start=False, stop=True,
                                     skip_group_check=True)
                ha4 = hap.tile([128, 4, T], bf16, name="ha4", tag="ha4")
                hin = AP(hp4[:].tensor, hp4[:].offset,
                         [(hp4[:].ap[0][0], 128), (512, 2), (T, 2), (1, T)])
                nc.scalar.activation(ha4[:], hin, AF.Prelu,
                                     bias=0.0, scale=1.0, alpha=0.25)

                def emit_sc(blk=blk, H=H, pos0=pos0, ha4=ha4):
                    for q in range(2):
                        k = pos0 // 2 + q          # pair index in half
                        nc.tensor.matmul(
                            s_ps[blk][64 * H : 64 * H + 64, 0 : 2 * T],
                            wq_t[:, k, :], ha4[:, 2 * q : 2 * q + 2, :],
                            tile_position=(0, 64 * H),
                            start=False, stop=(pos0 + 2 * q == 62),
                            skip_group_check=True,
                        )
                pending_sc.append(emit_sc)
                if len(pending_sc) > 1:
                    pending_sc.pop(0)()

        def softmax_half(hidx):
            blk, H = hidx // 2, hidx % 2
            base = 64 * H
            pn = halfp.tile([128, T], bf16, name="pn", tag="pn")
            for sslot in range(2):
                sps = s_ps[blk][base : base + 64, sslot * T : (sslot + 1) * T]
                msk = mf_s[blk][base : base + 64, :]
                smt = halfp.tile([128, T], fp32, name="smt", tag=f"smt{sslot}")
                nc.vector.scalar_tensor_tensor(
                    smt[base : base + 64, :], sps, BIG, msk,
                    op0=Alu.add, op1=Alu.mult,
                )
                # no max-subtraction: scores are O(30) so exp fits fp32;
                # masked entries exp(-BIG) underflow to exactly 0, so the
                # Exp activation's accum_out is already the masked sum
                p_t = halfp.tile([128, T], bf16, name="p_t", tag=f"p_t{sslot}")
                den = smallp.tile([128, 1], fp32, name="den", tag=f"den{sslot}")
                nc.scalar.activation(p_t[base : base + 64, :],
                                     smt[base : base + 64, :], AF.Exp,
                                     bias=nbig_t[base : base + 64, :],
                                     scale=1.0,
                                     accum_out=den[base : base + 64, :])
                denc = smallp.tile([128, 1], fp32, name="denc",
                                   tag=f"denc{sslot}")
                nc.vector.tensor_scalar_max(
                    denc[base : base + 64, :], den[base : base + 64, :], 1e-30)
                rec = smallp.tile([128, 1], fp32, name="rec", tag=f"rec{sslot}")
                nc.vector.reciprocal(
                    rec[base : base + 64, :], denc[base : base + 64, :])
                lo = base + 32 * sslot
                nc.scalar.mul(pn[lo : lo + 32, :], p_t[lo : lo + 32, :],
                              rec[lo : lo + 32, :])
            # bounce through DRAM. The write goes on the idle GPSIMD SWDGE
            # ring where waiting for the scalar muls costs nothing; the
            # partition-broadcast read-back is emitted later (bcast_half)
            # so it never stalls the sync queue head. The LAST half's write
            # goes on the (empty by then) sync ring instead -- HWDGE has a
            # much shorter write-receipt latency and this chain is the tail.
            pnd = dramp.tile([64, T], bf16, name="pnd", tag="pnd")
            ring = nc.sync if hidx == NHALF - 1 else nc.gpsimd
            ring.dma_start(pnd[:], pn[base : base + 64, :])
            return pnd

        def bcast_half(pnd):
            # pb[p, r, t] = pn_half[r, t] for all p
            pb = pbp.tile([128, 64, T], bf16, name="pb", tag="pb")
            src = AP(pnd[:].tensor, pnd[:].offset, [(0, 128), (T, 64), (1, T)])
            nc.sync.dma_start(pb[:], src)
            return pb

        def reduce_chunk(hidx, pb, c_h):
            # rows pos_h = 16*c_h + i, i=0..15; within-half row index
            # r = 32*(i%2) + 8*c_h + i//2  ->  src1 dims (a=i//2, b=i%2, t)
            g = 4 * hidx + c_h
            ktt = kts[g]
            pb_ap = pb[:]
            src1 = AP(pb_ap.tensor, pb_ap.offset + 8 * c_h * T,
                      [(pb_ap.ap[0][0], 128), (T, 8), (32 * T, 2), (1, T)])
            jk = jkp.tile([128, 16, T], bf16, name="jk", tag="jk")
            nc.vector.tensor_tensor(jk[:], ktt[:], src1, op=Alu.mult)
            # pairwise fold tree in 2x bf16 mode before the 1x reduce:
            # 200 -> 100 -> 50 elements per row
            jk2 = jkp.tile([128, 16, 100], bf16, name="jk2", tag="jk2")
            nc.vector.tensor_add(jk2[:], jk[:, :, 0:100], jk[:, :, 100:200])
            jk3 = jkp.tile([128, 16, 50], bf16, name="jk3", tag="jk3")
            nc.vector.tensor_add(jk3[:], jk2[:, :, 0:50], jk2[:, :, 50:100])
            nc.vector.tensor_reduce(
                vts[hidx][:, 16 * c_h : 16 * c_h + 16], jk3[:],
                mybir.AxisListType.X, Alu.add)

        def out_half(hidx):
            # contiguous processing-order output; waits freely on the
            # GPSIMD SWDGE ring (host undoes the row permutation)
            nc.gpsimd.dma_start(out[hidx], vts[hidx][:])

        # per-mlp-chunk reduce schedule for the previous half: the
        # broadcast lands during chunk 0, so reduction chunks run
        # {c1: [0, 1], c2: [2], c3: [3]}
        RSCHED = {1: [0, 1], 2: [2], 3: [3]}

        pnds = [None] * NHALF
        pbs = [None] * NHALF
        for hidx in range(NHALF):
            for c_h in range(4):
                extra = None
                if hidx > 0:
                    if c_h == 0:
                        # pn(hidx-1) is already in flight on the gpsimd
                        # ring; the broadcast read-back waits only briefly
                        # at the sync queue head
                        pbs[hidx - 1] = bcast_half(pnds[hidx - 1])
                    pidx, rcs = hidx - 1, RSCHED.get(c_h, [])
                    extra = (lambda p=pidx, cs=rcs:
                             [reduce_chunk(p, pbs[p], c) for c in cs])
                mlp_chunk(hidx, c_h, extra)
            flush_sc()
            pnds[hidx] = softmax_half(hidx)
            if hidx > 0:
                out_half(hidx - 1)
        pbs[NHALF - 1] = bcast_half(pnds[NHALF - 1])
        for c_h in range(4):
            reduce_chunk(NHALF - 1, pbs[NHALF - 1], c_h)
        out_half(NHALF - 1)

    nc.compile()
    return nc


def _prep_inputs(query, keys, mask, w1, b1, prelu_a, w2, b2):
    """Host-side restaging of the full inputs into per-core DMA-friendly
    layouts. Returns list of per-core input maps."""
    query = np.asarray(query, dtype=np.float32)
    keys = np.asarray(keys, dtype=np.float32)
    mask = np.asarray(mask)
    w1 = np.asarray(w1, dtype=np.float32)
    b1 = np.asarray(b1, dtype=np.float32)
    w2 = np.asarray(w2, dtype=np.float32)
    b2 = np.asarray(b2, dtype=np.float32)
    alpha = float(np.asarray(prelu_a))
    assert abs(alpha - 0.25) < 1e-9, "kernel hardcodes PReLU slope 0.25"

    Wq, Wk, Wc, Wd = (w1[:, :D], w1[:, D : 2 * D], w1[:, 2 * D : 3 * D],
                      w1[:, 3 * D :])
    wa = np.ascontiguousarray((Wk - Wc).T)                      # [d_in, d_out]
    wd = np.ascontiguousarray(Wd.T)                             # [d_in, d_out]
    bias = (query @ (Wq + Wc).T + b1).astype(BF16)              # [B, D]
    w2c = w2[:, 0].astype(BF16)                                 # [D]
    wq = np.zeros((D, 32, 64), dtype=BF16)
    for c in range(32):
        wq[:, c, c] = w2c
        wq[:, c, 32 + c] = w2c
    ey = np.eye(D, 128, dtype=BF16)

    # host-folded W_b per row: wb[r] = wa + q[r, d_in] * wd
    wb_all = (wa[None, :, :] + query[:, :, None] * wd[None, :, :]).astype(BF16)

    keys_bf = keys.astype(BF16)                                  # [B, T, D]
    mfull = mask.astype(BF16)

    # processing-order permutation: half-major, LBH within each half
    order = np.concatenate(
        [64 * h + np.asarray(LBH) for h in range(NHALF)]
    )

    in_maps = []
    for c in range(NCORES):
        s = slice(c * BC, (c + 1) * BC)
        kt_rows = keys_bf[s][order].transpose(0, 2, 1)           # [256, D, T]
        ktg = np.ascontiguousarray(
            kt_rows.reshape(NCH, 16, D, T).transpose(0, 2, 1, 3)
        )                                                        # [16, D, 16, T]
        wbg = np.ascontiguousarray(
            wb_all[s][order].reshape(NCH, 16, D, D).transpose(0, 2, 1, 3)
        )                                                        # [16, D, 16, D]
        btv = np.ascontiguousarray(
            bias[s][order].reshape(2, 128, D)
        )                                                        # [2, 128, D]
        in_maps.append(
            {
                "ktg": ktg,
                "wbg": wbg,
                "mf": np.ascontiguousarray(mfull[s]),
                "btn": btv,
                "wq": wq,
                "ey": ey,
            }
        )
    return in_maps


def _get_module():
    if "module" not in _CACHE:
        _CACHE["module"] = _build_module()
    return _CACHE["module"]


def kernel(query, keys, mask, w1, b1, prelu_a, w2, b2):
    from concourse.bass_utils import run_bass_kernel_spmd

    nc = _get_module()
    in_maps = _prep_inputs(query, keys, mask, w1, b1, prelu_a, w2, b2)
    res = run_bass_kernel_spmd(nc, in_maps, list(range(NCORES)))
    _CACHE["last_results"] = res
    # per-core out: [NHALF, D, 64] processing order -> [BC, D] natural
    order = np.concatenate([64 * h + np.asarray(LBH) for h in range(NHALF)])
    inv = np.empty(BC, dtype=np.int64)
    inv[order] = np.arange(BC)
    outs = []
    for r in res.results:
        o = r["out"].transpose(0, 2, 1).reshape(BC, D)   # processing order
        outs.append(o[inv])
    out = np.concatenate(outs, axis=0)
    return out.astype(np.float32)


# revision 27
# speedup vs baseline: 1.1501x; 1.1501x over previous
"""DIN activation unit kernel for 8x TRN2 NeuronCores (V2.2).

Math (per batch row b, per key position t):
  h[t]  = W_b @ k[t] + c_b,  W_b = (Wk-Wc) + Wd*diag(q_b)  (host-folded)
  s[t]  = w2 . PReLU(h[t], 0.25)
  p     = softmax over masked t;  w = p*mask / max(sum, 1e-6)
  out   = sum_t w[t] * k[t]

Pipeline: 4 half-blocks of 64 rows; each half's scores accumulate in its
own 64-partition PSUM region (own stop flag) so softmax + weighted
reduction of half X overlap the MLP of half X+1.

Per row pair the PSUM tile is seeded with the bias columns by one
selector matmul (stationary = natural-layout bias block, moving = two
eye columns stride-0-replicated across t), the two h matmuls accumulate
on top, and ONE PReLU activation (no bias) covers both rows -- halving
the scalar-engine instruction count, the old wall.

The weighted reduction runs as one bf16 2x-mode tensor_tensor
(kt * broadcast weights) + one tensor_reduce per 16-row chunk. Softmax
weights reach all 128 partitions via DRAM bounce (write on the scalar
HWDGE ring, stride-0 partition-broadcast read on the sync ring).
Softmax skips max-subtraction (scores are small; exp stays in fp32
range). kt/W_b stream on the sync ring in 16-row chunks.
"""

import numpy as np
import ml_dtypes

B, T, D = 2048, 200, 128
NCORES = 8
BC = B // NCORES          # 256 batch rows per core
NHALF = 4                 # 64-row half-blocks per core
NCH = 16                  # 16-row chunks per core (4 per half)
BF16 = ml_dtypes.bfloat16
BIG = 1024.0              # mask shift; exp(-~1024) == 0 in fp32

# processing order within a 64-row half: pairs (k, k+32) so one
# double-diag scores matmul serves both rows
LBH = [32 * (i % 2) + i // 2 for i in range(64)]

_CACHE = {}


def _build_module():
    from contextlib import ExitStack

    import concourse.bacc as bacc
    import concourse.mybir as mybir
    from concourse import tile
    from concourse.bass import AP

    fp32 = mybir.dt.float32
    bf16 = mybir.dt.bfloat16
    Alu = mybir.AluOpType
    AF = mybir.ActivationFunctionType

    nc = bacc.Bacc(
        "TRN2", target_bir_lowering=False, debug=False, num_devices=NCORES
    )

    # transposed keys, [d, t] per row, rows processing-ordered, 16-row chunks
    kt_d = nc.dram_tensor("ktg", [NCH, D, 16, T], bf16, kind="ExternalInput")
    # host-folded W_b per row, [d_in, d_out], same order/chunking
    wb_d = nc.dram_tensor("wbg", [NCH, D, 16, D], bf16, kind="ExternalInput")
    mf_d = nc.dram_tensor("mf", [BC, T], bf16, kind="ExternalInput")
    # bias rows in processing order, natural [row, d] layout
    bt_d = nc.dram_tensor("btn", [2, 128, D], bf16, kind="ExternalInput")
    # precomputed scores selector: wq[d, c, c] = wq[d, c, 32+c] = w2[d]
    wq_d = nc.dram_tensor("wq", [D, 32, 64], bf16, kind="ExternalInput")
    ey_d = nc.dram_tensor("ey", [D, 128], bf16, kind="ExternalInput")
    # processing-order output [half, d, pos]; host undoes the permutation
    out_d = nc.dram_tensor("out", [NHALF, D, 64], fp32, kind="ExternalOutput")

    kt = kt_d.ap()
    wb = wb_d.ap()
    mf = mf_d.ap()
    bt = bt_d.ap()
    out = out_d.ap()

    with ExitStack() as ctx:
        tc = ctx.enter_context(tile.TileContext(nc))
        const = ctx.enter_context(tc.tile_pool(name="const", bufs=1))
        ktp = ctx.enter_context(tc.tile_pool(name="ktp", bufs=12))
        wbp = ctx.enter_context(tc.tile_pool(name="wbp", bufs=5))
        hap = ctx.enter_context(tc.tile_pool(name="hap", bufs=6))
        blkp = ctx.enter_context(tc.tile_pool(name="blkp", bufs=2))
        halfp = ctx.enter_context(tc.tile_pool(name="halfp", bufs=2))
        smallp = ctx.enter_context(tc.tile_pool(name="smallp", bufs=4))
        pbp = ctx.enter_context(tc.tile_pool(name="pbp", bufs=2))
        jkp = ctx.enter_context(tc.tile_pool(name="jkp", bufs=2))
        dramp = ctx.enter_context(tc.tile_pool(name="dramp", bufs=2,
                                               space="DRAM"))
        hpp = ctx.enter_context(tc.tile_pool(name="hpp", bufs=3, space="PSUM"))
        spp = ctx.enter_context(tc.tile_pool(name="spp", bufs=2, space="PSUM"))

        kts = [None] * NCH
        wbs = [None] * NCH

        def load_chunk(g):
            ktt = ktp.tile([D, 16, T], bf16, name="ktt", tag="kt")
            nc.sync.dma_start(ktt[:], kt[g])
            kts[g] = ktt
            wbt = wbp.tile([D, 16, D], bf16, name="wbt", tag="wb")
            nc.sync.dma_start(wbt[:], wb[g])
            wbs[g] = wbt

        load_chunk(0)
        load_chunk(1)

        bt_s, mf_s = [], []
        wq_t = const.tile([D, 32, 64], bf16, name="wq_t")
        nc.sync.dma_start(wq_t[:], wq_d.ap()[:])
        ey_t = const.tile([D, 128], bf16, name="ey_t")
        nc.sync.dma_start(ey_t[:], ey_d.ap()[:])
        for blk in range(2):
            bs = blkp.tile([128, D], bf16, name="bt_s", tag="bt_s")
            nc.sync.dma_start(bs[:], bt[blk])
            bt_s.append(bs)
            ms = blkp.tile([128, T], bf16, name="mf_s", tag="mf_s")
            nc.sync.dma_start(ms[:], mf[blk * 128 : (blk + 1) * 128, :])
            mf_s.append(ms)

        # zero tiles for the s_ps has_written init matmuls
        zw_t = const.tile([D, 64], bf16, name="zw_t")
        nc.gpsimd.memset(zw_t[:], 0.0)
        nbig_t = const.tile([128, 1], fp32, name="nbig_t")
        nc.gpsimd.memset(nbig_t[:], -BIG)
        zr_t = const.tile([D, 2 * T], bf16, name="zr_t")
        nc.gpsimd.memset(zr_t[:], 0.0)
        s_ps = []
        for blk in range(2):
            sp = spp.tile([128, 512], fp32, name="s_ps", tag="s_ps")
            for H in range(2):
                nc.tensor.matmul(sp[64 * H : 64 * H + 64, 0 : 2 * T],
                                 zw_t[:], zr_t[:],
                                 tile_position=(0, 64 * H),
                                 start=True, stop=False, skip_group_check=True)
            s_ps.append(sp)

        # per-half output accumulators [d, pos] (processing order)
        vts = [const.tile([D, 64], fp32, name=f"vt_{h}") for h in range(NHALF)]

        ey_ap = ey_t[:]
        ey_pstep = ey_ap.ap[0][0]

        # scores matmuls deferred by one quad so the PE never sits behind
        # the activation of the quad it just produced
        pending_sc = []

        def flush_sc():
            while pending_sc:
                pending_sc.pop(0)()

        def mlp_chunk(hidx, c_h, extra=None):
            blk, H = hidx // 2, hidx % 2
            g = 4 * hidx + c_h
            if g + 2 < NCH and kts[g + 2] is None:
                load_chunk(g + 2)
            if extra is not None:
                extra()
            ktt, wbt = kts[g], wbs[g]
            for i4 in range(4):                    # quad of 4 rows
                pos0 = 16 * c_h + 4 * i4           # within-half quad base
                posb = 64 * H + pos0               # block-wise row
                # two-bank PSUM tile; row j lives at columns
                # {0:200, 200:400, 512:712, 712:912} so each matmul's
                # output stays inside one bank
                hp4 = hpp.tile([128, 1024], fp32, name="hp4", tag="hp")
                # seed PSUM with bias columns: stationary is the natural
                # bias block, moving = eye cols stride-0-replicated T times
                for q in range(2):
                    esel = AP(ey_ap.tensor, ey_ap.offset + posb + 2 * q,
                              [(ey_pstep, 128), (1, 2), (0, T)])
                    nc.tensor.matmul(hp4[:, 512 * q : 512 * q + 2 * T],
                                     bt_s[blk][:], esel,
                                     start=True, stop=False,
                                     skip_group_check=True)
                for j in range(4):
                    i = 4 * i4 + j
                    lo = 512 * (j // 2) + T * (j % 2)
                    nc.tensor.matmul(hp4[:, lo : lo + T], wbt[:, i, :],
                                     ktt[:, i, :],
                                     start=False, stop=True,
                                     skip_group_check=True)
                ha4 = hap.tile([128, 4, T], bf16, name="ha4", tag="ha4")
                hin = AP(hp4[:].tensor, hp4[:].offset,
                         [(hp4[:].ap[0][0], 128), (512, 2), (T, 2), (1, T)])
                nc.scalar.activation(ha4[:], hin, AF.Prelu,
                                     bias=0.0, scale=1.0, alpha=0.25)

                for q in range(2):
                    k = pos0 // 2 + q              # pair index in half
                    nc.tensor.matmul(
                        s_ps[blk][64 * H : 64 * H + 64, 0 : 2 * T],
                        wq_t[:, k, :], ha4[:, 2 * q : 2 * q + 2, :],
                        tile_position=(0, 64 * H),
                        start=False, stop=(pos0 + 2 * q == 62),
                        skip_group_check=True,
                    )

        def softmax_half(hidx):
            blk, H = hidx // 2, hidx % 2
            base = 64 * H
            pn = halfp.tile([128, T], bf16, name="pn", tag="pn")
            for sslot in range(2):
                sps = s_ps[blk][base : base + 64, sslot * T : (sslot + 1) * T]
                msk = mf_s[blk][base : base + 64, :]
                smt = halfp.tile([128, T], fp32, name="smt", tag=f"smt{sslot}")
                nc.vector.scalar_tensor_tensor(
                    smt[base : base + 64, :], sps, BIG, msk,
                    op0=Alu.add, op1=Alu.mult,
                )
                # no max-subtraction: scores are O(30) so exp fits fp32;
                # masked entries exp(-BIG) underflow to exactly 0, so the
                # Exp activation's accum_out is already the masked sum
                p_t = halfp.tile([128, T], bf16, name="p_t", tag=f"p_t{sslot}")
                den = smallp.tile([128, 1], fp32, name="den", tag=f"den{sslot}")
                nc.scalar.activation(p_t[base : base + 64, :],
                                     smt[base : base + 64, :], AF.Exp,
                                     bias=nbig_t[base : base + 64, :],
                                     scale=1.0,
                                     accum_out=den[base : base + 64, :])
                denc = smallp.tile([128, 1], fp32, name="denc",
                                   tag=f"denc{sslot}")
                nc.vector.tensor_scalar_max(
                    denc[base : base + 64, :], den[base : base + 64, :], 1e-30)
                rec = smallp.tile([128, 1], fp32, name="rec", tag=f"rec{sslot}")
                nc.vector.reciprocal(
                    rec[base : base + 64, :], denc[base : base + 64, :])
                lo = base + 32 * sslot
                nc.scalar.mul(pn[lo : lo + 32, :], p_t[lo : lo + 32, :],
                              rec[lo : lo + 32, :])
            # bounce through DRAM. The write goes on the idle GPSIMD SWDGE
            # ring where waiting for the scalar muls costs nothing; the
            # partition-broadcast read-back is emitted later (bcast_half)
            # so it never stalls the sync queue head. The LAST half's write
            # goes on the (empty by then) sync ring instead -- HWDGE has a
            # much shorter write-receipt latency and this chain is the tail.
            pnd = dramp.tile([64, T], bf16, name="pnd", tag="pnd")
            ring = nc.sync if hidx == NHALF - 1 else nc.gpsimd
            ring.dma_start(pnd[:], pn[base : base + 64, :])
            return pnd

        def bcast_half(pnd):
            # pb[p, r, t] = pn_half[r, t] for all p
            pb = pbp.tile([128, 64, T], bf16, name="pb", tag="pb")
            src = AP(pnd[:].tensor, pnd[:].offset, [(0, 128), (T, 64), (1, T)])
            nc.sync.dma_start(pb[:], src)
            return pb

        def reduce_chunk(hidx, pb, c_h):
            # rows pos_h = 16*c_h + i, i=0..15; within-half row index
            # r = 32*(i%2) + 8*c_h + i//2  ->  src1 dims (a=i//2, b=i%2, t)
            g = 4 * hidx + c_h
            ktt = kts[g]
            pb_ap = pb[:]
            src1 = AP(pb_ap.tensor, pb_ap.offset + 8 * c_h * T,
                      [(pb_ap.ap[0][0], 128), (T, 8), (32 * T, 2), (1, T)])
            jk = jkp.tile([128, 16, T], bf16, name="jk", tag="jk")
            nc.vector.tensor_tensor(jk[:], ktt[:], src1, op=Alu.mult)
            # pairwise fold tree in 2x bf16 mode before the 1x reduce:
            # 200 -> 100 -> 50 elements per row
            jk2 = jkp.tile([128, 16, 100], bf16, name="jk2", tag="jk2")
            nc.vector.tensor_add(jk2[:], jk[:, :, 0:100], jk[:, :, 100:200])
            jk3 = jkp.tile([128, 16, 50], bf16, name="jk3", tag="jk3")
            nc.vector.tensor_add(jk3[:], jk2[:, :, 0:50], jk2[:, :, 50:100])
            nc.vector.tensor_reduce(
                vts[hidx][:, 16 * c_h : 16 * c_h + 16], jk3[:],
                mybir.AxisListType.X, Alu.add)

        def out_half(hidx):
            # contiguous processing-order output; waits freely on the
            # GPSIMD SWDGE ring (host undoes the row permutation)
            nc.gpsimd.dma_start(out[hidx], vts[hidx][:])

        # per-mlp-chunk reduce schedule for the previous half: the
        # broadcast lands during chunk 0, so reduction chunks run
        # {c1: [0, 1], c2: [2], c3: [3]}
        RSCHED = {1: [0, 1], 2: [2], 3: [3]}

        pnds = [None] * NHALF
        pbs = [None] * NHALF
        for hidx in range(NHALF):
            for c_h in range(4):
                extra = None
                if hidx > 0:
                    if c_h == 0:
                        # pn(hidx-1) is already in flight on the gpsimd
                        # ring; the broadcast read-back waits only briefly
                        # at the sync queue head
                        pbs[hidx - 1] = bcast_half(pnds[hidx - 1])
                    pidx, rcs = hidx - 1, RSCHED.get(c_h, [])
                    extra = (lambda p=pidx, cs=rcs:
                             [reduce_chunk(p, pbs[p], c) for c in cs])
                mlp_chunk(hidx, c_h, extra)
            flush_sc()
            pnds[hidx] = softmax_half(hidx)
            if hidx > 0:
                out_half(hidx - 1)
        pbs[NHALF - 1] = bcast_half(pnds[NHALF - 1])
        for c_h in range(4):
            reduce_chunk(NHALF - 1, pbs[NHALF - 1], c_h)
        out_half(NHALF - 1)

    nc.compile()
    return nc


def _prep_inputs(query, keys, mask, w1, b1, prelu_a, w2, b2):
    """Host-side restaging of the full inputs into per-core DMA-friendly
    layouts. Returns list of per-core input maps."""
    query = np.asarray(query, dtype=np.float32)
    keys = np.asarray(keys, dtype=np.float32)
    mask = np.asarray(mask)
    w1 = np.asarray(w1, dtype=np.float32)
    b1 = np.asarray(b1, dtype=np.float32)
    w2 = np.asarray(w2, dtype=np.float32)
    b2 = np.asarray(b2, dtype=np.float32)
    alpha = float(np.asarray(prelu_a))
    assert abs(alpha - 0.25) < 1e-9, "kernel hardcodes PReLU slope 0.25"

    Wq, Wk, Wc, Wd = (w1[:, :D], w1[:, D : 2 * D], w1[:, 2 * D : 3 * D],
                      w1[:, 3 * D :])
    wa = np.ascontiguousarray((Wk - Wc).T)                      # [d_in, d_out]
    wd = np.ascontiguousarray(Wd.T)                             # [d_in, d_out]
    bias = (query @ (Wq + Wc).T + b1).astype(BF16)              # [B, D]
    w2c = w2[:, 0].astype(BF16)                                 # [D]
    wq = np.zeros((D, 32, 64), dtype=BF16)
    for c in range(32):
        wq[:, c, c] = w2c
        wq[:, c, 32 + c] = w2c
    ey = np.eye(D, 128, dtype=BF16)

    # host-folded W_b per row: wb[r] = wa + q[r, d_in] * wd
    wb_all = (wa[None, :, :] + query[:, :, None] * wd[None, :, :]).astype(BF16)

    keys_bf = keys.astype(BF16)                                  # [B, T, D]
    mfull = mask.astype(BF16)

    # processing-order permutation: half-major, LBH within each half
    order = np.concatenate(
        [64 * h + np.asarray(LBH) for h in range(NHALF)]
    )

    in_maps = []
    for c in range(NCORES):
        s = slice(c * BC, (c + 1) * BC)
        kt_rows = keys_bf[s][order].transpose(0, 2, 1)           # [256, D, T]
        ktg = np.ascontiguousarray(
            kt_rows.reshape(NCH, 16, D, T).transpose(0, 2, 1, 3)
        )                                                        # [16, D, 16, T]
        wbg = np.ascontiguousarray(
            wb_all[s][order].reshape(NCH, 16, D, D).transpose(0, 2, 1, 3)
        )                                                        # [16, D, 16, D]
        btv = np.ascontiguousarray(
            bias[s][order].reshape(2, 128, D)
        )                                                        # [2, 128, D]
        in_maps.append(
            {
                "ktg": ktg,
                "wbg": wbg,
                "mf": np.ascontiguousarray(mfull[s]),
                "btn": btv,
                "wq": wq,
                "ey": ey,
            }
        )
    return in_maps


def _get_module():
    if "module" not in _CACHE:
        _CACHE["module"] = _build_module()
    return _CACHE["module"]


def kernel(query, keys, mask, w1, b1, prelu_a, w2, b2):
    from concourse.bass_utils import run_bass_kernel_spmd

    nc = _get_module()
    in_maps = _prep_inputs(query, keys, mask, w1, b1, prelu_a, w2, b2)
    res = run_bass_kernel_spmd(nc, in_maps, list(range(NCORES)))
    _CACHE["last_results"] = res
    # per-core out: [NHALF, D, 64] processing order -> [BC, D] natural
    order = np.concatenate([64 * h + np.asarray(LBH) for h in range(NHALF)])
    inv = np.empty(BC, dtype=np.int64)
    inv[order] = np.arange(BC)
    outs = []
    for r in res.results:
        o = r["out"].transpose(0, 2, 1).reshape(BC, D)   # processing order
        outs.append(o[inv])
    out = np.concatenate(outs, axis=0)
    return out.astype(np.float32)
